# revision 5
# baseline (speedup 1.0000x reference)
"""DeeperGCN (GENConv x4) forward on 8 Trainium2 NeuronCores — v4.

v4 over the v2 baseline (4.40ms -> 2.53ms on HW):
  - dma_gather descriptor generation spread over all 4 SWDGE queues
    (num_swdge_queues=4): each queue runs on its own Q7 core pair, so the
    four per-chunk gathers of a group overlap (~3x desc-gen throughput).
    reg_loads hoisted and gathers issued back-to-back per group.
  - per-(group,chunk,window) slot runs use offsets COMMON across cores
    (padded to the max core count), so scatter tile spans are identical on
    every core: fewer scatter matmuls, no cross-core union widening.
  - message relu moved to the Scalar engine; eps*deg folded into the
    X-add via scalar_tensor_tensor (eps_sb), cutting the DVE message pass
    from 2 ops to 1.
  - LayerNorm tail batched per window-group: sqrt/reciprocal/neg-mean are
    computed for 4 windows at once; the LN apply reads y1 directly from
    PSUM (no SBUF staging copy); y1 for the group lives in one 2-bank
    PSUM tile.
"""
import numpy as np

H = 128
L = 4
EPS_MSG = 1e-7
EPS_LN = 1e-5


class CFG:
    def __init__(self, n_nodes=100000, n_graphs=512, n_cores=8, win=128,
                 gw=4, nchunk=4):
        self.N = n_nodes
        self.G = n_graphs
        self.NC = n_cores
        self.SH = n_nodes // n_cores
        self.WIN = win
        self.SHP = ((self.SH + win - 1) // win) * win
        self.NW = self.SHP // win
        self.NCHUNK = nchunk
        assert (self.NC * self.SHP) % nchunk == 0
        self.CH = self.NC * self.SHP // nchunk
        assert self.CH <= 32767, "int16 gather index limit"
        self.GW = gw
        self.NGRP = (self.NW + gw - 1) // gw

    def grp_windows(self, g):
        return min(self.GW, self.NW - g * self.GW)


class Plan:
    """Static (core-independent) packing plan with per-(g,q,wg) run offsets
    COMMON across cores: each window-chunk run is padded to the max count
    over cores, so the scatter tile spans are identical on every core (no
    cross-core union widening)."""

    def __init__(self, cfg, counts3):
        # counts3: [NC, NGRP, NCHUNK, GW] real edge counts per window-run
        c = cfg
        maxc = counts3.max(axis=0)                      # [NGRP, NCHUNK, GW]
        self.maxc = maxc
        self.run_off = np.zeros((c.NGRP, c.NCHUNK, c.GW), np.int64)
        self.run_off[:, :, 1:] = np.cumsum(maxc, axis=2)[:, :, :-1]
        sec_len = maxc.sum(axis=2)                      # [NGRP, NCHUNK]
        self.T = np.maximum(1, (sec_len + 127) // 128)  # tiles/section
        self.sec_tile_base = np.zeros((c.NGRP, c.NCHUNK), np.int64)
        t = 0
        for g in range(c.NGRP):
            for q in range(c.NCHUNK):
                self.sec_tile_base[g, q] = t
                t += self.T[g, q]
        self.NTILES = int(t)
        self.NSLOT = self.NTILES * 128
        self.key = ((c.N, c.G, c.NC, c.GW, c.NCHUNK)
                    + tuple(self.T.reshape(-1))
                    + tuple(self.run_off.reshape(-1)))


def bucket_core(cfg, core, src, dst, attr):
    """Per-core edges bucketed by (group, chunk), sorted by (window, crow)."""
    c = cfg
    sel = (dst // c.SH) == core
    s, d, a = src[sel], dst[sel], attr[sel]
    local = d - core * c.SH
    win = local // c.WIN
    dst_rel = local % c.WIN
    # table rows are partition-major: row = core*SHP + p*NW + w (so the
    # per-layer t_sb -> t_stage staging DMA is contiguous per partition)
    s_loc = s % c.SH
    pad_row = (s // c.SH) * c.SHP + (s_loc % c.WIN) * c.NW + s_loc // c.WIN
    chunk = pad_row // c.CH
    crow = pad_row % c.CH
    grp = win // c.GW
    order = np.lexsort((crow, win, chunk, grp))
    return (grp[order], chunk[order], win[order], dst_rel[order],
            crow[order], a[order])


def build_plan(cfg, bucketed):
    c = cfg
    counts3 = np.zeros((c.NC, c.NGRP, c.NCHUNK, c.GW), np.int64)
    for core in range(c.NC):
        g_, q_, w_, _, _, _ = bucketed[core]
        np.add.at(counts3, (core, g_, q_, w_ - g_ * c.GW), 1)
    return Plan(cfg, counts3), counts3


def build_smat_schedule(cfg, plan, bucketed, counts3):
    """Static scatter schedule from the COMMON run layout: per (g, wg) a
    list of (q, tile_in_section, smat_slot)."""
    c, p = cfg, plan
    sched = {}   # (g, wg) -> list of (q, tile, smat_slot)
    nsmat = 0
    grp_smat_base = []
    for g in range(c.NGRP):
        grp_smat_base.append(nsmat)
        for wg in range(c.grp_windows(g)):
            lst = []
            for q in range(c.NCHUNK):
                mc = int(p.maxc[g, q, wg])
                if mc == 0:
                    continue
                lo = int(p.run_off[g, q, wg]) // 128
                hi = int(p.run_off[g, q, wg] + mc - 1) // 128
                for t in range(lo, hi + 1):
                    lst.append((q, t, nsmat))
                    nsmat += 1
            sched[(g, wg)] = lst
    return sched, nsmat, grp_smat_base


def prep_core(cfg, plan, sched, core, bucketed, batch, Etab):
    c, p = cfg, plan
    g_, q_, w_, rel_, cr_, a_ = bucketed[core]
    nkey3 = c.NGRP * c.NCHUNK * c.GW
    key3 = (g_ * c.NCHUNK + q_) * c.GW + (w_ - g_ * c.GW)
    bc3 = np.bincount(key3, minlength=nkey3).reshape(
        c.NGRP, c.NCHUNK, c.GW)

    # interior pads gather row 0 (S zeroes them); section tails are -1
    # (trimmed by the ucode below the cnt register value)
    slot_src = np.full(p.NSLOT, 0, np.int16)
    slot_rel = np.full(p.NSLOT, -1, np.int32)
    slot_attr = np.full(p.NSLOT, -1, np.int32)
    slot_w = np.full(p.NSLOT, -1, np.int32)
    cnts = np.zeros(c.NGRP * c.NCHUNK, np.int32)
    pos = 0
    for g in range(c.NGRP):
        for q in range(c.NCHUNK):
            base = p.sec_tile_base[g, q] * 128
            cap = int(p.T[g, q]) * 128
            last_end = 0
            for wg in range(c.GW):
                cnt = int(bc3[g, q, wg])
                if cnt == 0:
                    continue
                off = int(p.run_off[g, q, wg])
                sl = slice(pos, pos + cnt)
                slot_src[base + off:base + off + cnt] = cr_[sl]
                slot_rel[base + off:base + off + cnt] = rel_[sl]
                slot_attr[base + off:base + off + cnt] = a_[sl]
                slot_w[base + off:base + off + cnt] = wg
                pos += cnt
                last_end = off + cnt
            cnts[g * c.NCHUNK + q] = last_end
            slot_src[base + last_end:base + cap] = -1
    assert pos == len(g_)

    # idx buffers wrapped in 16 partitions, one call per (g, q)
    cols = []
    for g in range(c.NGRP):
        for q in range(c.NCHUNK):
            nidx = int(p.T[g, q]) * 128
            base = p.sec_tile_base[g, q] * 128
            lst = slot_src[base:base + nidx]
            arr = np.empty((128, nidx // 16), np.int16)
            cidx = np.arange(nidx // 16) * 16
            for pp in range(128):
                arr[pp, :] = lst[cidx + (pp % 16)]
            cols.append(arr)
    idx_buf = np.ascontiguousarray(np.concatenate(cols, axis=1))

    # S data per smat slot: [NSMAT, 128, WIN]
    nsmat = max(s for lst in sched.values() for (_, _, s) in lst) + 1
    S = np.zeros((nsmat, 128, c.WIN), np.float32)
    rel2 = slot_rel.reshape(p.NTILES, 128)
    w2 = slot_w.reshape(p.NTILES, 128)
    for (g, wg), lst in sched.items():
        for (q, t, sm) in lst:
            ti = p.sec_tile_base[g, q] + t
            mask = (w2[ti] == wg) & (rel2[ti] >= 0)
            rows = np.where(mask)[0]
            S[sm, rows, rel2[ti][rows]] = 1.0

    ap = slot_attr.reshape(p.NTILES, 128)
    E_pre = np.where(ap[:, :, None] >= 0,
                     Etab[np.clip(ap, 0, Etab.shape[0] - 1)], 0.0)

    # eps*deg and batch-relative per window
    deg = np.zeros(c.SHP, np.float32)
    sel_rel = slot_rel >= 0
    # recompute local dst index per real slot
    # window-of-slot: grp*GW + slot_w ; local = win*128 + rel
    tile_of_slot = np.arange(p.NSLOT) // 128
    grp_of_tile = np.zeros(p.NTILES, np.int64)
    for g in range(c.NGRP):
        for q in range(c.NCHUNK):
            b = p.sec_tile_base[g, q]
            grp_of_tile[b:b + p.T[g, q]] = g
    win_of_slot = grp_of_tile[tile_of_slot] * c.GW + slot_w
    loc = win_of_slot[sel_rel] * 128 + slot_rel[sel_rel]
    np.add.at(deg, loc, 1.0)
    eps_pm = np.ascontiguousarray((EPS_MSG * deg).reshape(c.NW, 128).T)

    b = batch[core * c.SH:(core + 1) * c.SH]
    g0 = int(b[0])
    batch_rel = np.full(c.SHP, -1.0, np.float32)
    batch_rel[:c.SH] = (b - g0).astype(np.float32)
    assert batch_rel.max() < 128
    batch_pm = np.ascontiguousarray(batch_rel.reshape(c.NW, 128).T)

    cnts = cnts.astype(np.int32)   # [NGRP*NCHUNK] trim position per call
    return dict(idx_buf=idx_buf, S=S, E=E_pre, eps_pm=eps_pm,
                batch_pm=batch_pm, g0=g0, cnts=cnts)


def build_program(cfg, plan, sched, nsmat, grp_smat_base, trivial,
                  scratch=16384, tab_bf16=True, mlp_bufs=4):
    import concourse.bass as bass
    import concourse.bacc as bacc
    import concourse.mybir as mybir
    import concourse.tile as tile
    from concourse.masks import make_identity

    c, p = cfg, plan
    f32 = mybir.dt.float32
    bf16 = mybir.dt.bfloat16
    i16 = mybir.dt.int16
    AF = mybir.ActivationFunctionType
    OP = mybir.AluOpType

    nc = bacc.Bacc("TRN2", target_bir_lowering=False, debug=False,
                   num_devices=c.NC, dynamic_dma_scratch_size=scratch,
                   num_swdge_queues=min(4, c.NCHUNK))

    tdt = bf16 if tab_bf16 else f32
    Tmax = int(p.T.max())
    grp_tiles = [int(p.T[g].sum()) for g in range(c.NGRP)]
    GT = max(grp_tiles)
    grp_smat = [
        (grp_smat_base[g + 1] if g + 1 < c.NGRP else nsmat) - grp_smat_base[g]
        for g in range(c.NGRP)]
    GS = max(grp_smat)

    # ---- DRAM inputs ----
    xT = nc.dram_tensor("xT", [128, c.SHP], bf16, kind="ExternalInput")
    We_d = nc.dram_tensor("We", [128, H], bf16, kind="ExternalInput")
    W1_d = nc.dram_tensor("W1", [L, 128, 2 * H], bf16, kind="ExternalInput")
    W2_d = nc.dram_tensor("W2", [L, 2, 128, H], bf16, kind="ExternalInput")
    idx_d = nc.dram_tensor("idx", [128, p.NSLOT // 16], i16, kind="ExternalInput")
    S_d = nc.dram_tensor("S", [nsmat, 128, c.WIN], bf16, kind="ExternalInput")
    E_d = nc.dram_tensor("E", [p.NTILES, 128, H], bf16, kind="ExternalInput")
    eps_d = nc.dram_tensor("epsdeg", [128, c.NW], f32, kind="ExternalInput")
    bat_d = nc.dram_tensor("batchrel", [128, c.NW], f32, kind="ExternalInput")
    iota_d = nc.dram_tensor("iota", [128, 128], f32, kind="ExternalInput")
    ncalls = c.NGRP * c.NCHUNK
    cnt_d = nc.dram_tensor("cnt", [128, ncalls], mybir.dt.int32,
                           kind="ExternalInput")
    aff_d = None
    if not trivial:
        aff_d = {
            "gn": nc.dram_tensor("gn", [L, 128, H], f32, kind="ExternalInput"),
            "bn": nc.dram_tensor("bn", [L, 128, H], f32, kind="ExternalInput"),
            "g1": nc.dram_tensor("g1", [L, 128, 2 * H], f32, kind="ExternalInput"),
            "bb1": nc.dram_tensor("bb1", [L, 128, 2 * H], f32, kind="ExternalInput"),
            "b1": nc.dram_tensor("b1", [L, 128, 2 * H], f32, kind="ExternalInput"),
            "b2": nc.dram_tensor("b2", [L, 128, H], f32, kind="ExternalInput"),
            "be": nc.dram_tensor("be", [128, H], f32, kind="ExternalInput"),
        }
    out_d = nc.dram_tensor("partial", [128, H], f32, kind="ExternalOutput")

    with tile.TileContext(nc) as tc:
        with tc.tile_pool(name="const", bufs=1) as cpool, \
             tc.tile_pool(name="msg", bufs=4) as msgpool, \
             tc.tile_pool(name="emb", bufs=2) as epool, \
             tc.tile_pool(name="smat", bufs=2) as spool, \
             tc.tile_pool(name="mbf", bufs=3) as mbfpool, \
             tc.tile_pool(name="mlp", bufs=mlp_bufs) as mlppool, \
             tc.tile_pool(name="small", bufs=4) as smpool, \
             tc.tile_pool(name="psA", bufs=2, space="PSUM") as psA, \
             tc.tile_pool(name="psB", bufs=2, space="PSUM") as psB, \
             tc.tile_pool(name="psC", bufs=1, space="PSUM") as psC, \
             tc.tile_pool(name="psY", bufs=1, space="PSUM") as psY, \
             tc.tile_pool(name="psPool", bufs=1, space="PSUM") as psP, \
             tc.tile_pool(name="dram", bufs=1, space="DRAM") as dpool:

            # ---- persistent DRAM state ----
            t_stage = dpool.tile([c.SHP, H], tdt)
            t_fulls = []
            for l in range(L):
                tf = dpool.tile([c.NC * c.SHP, H], tdt, addr_space="Shared",
                                tag=f"t_full{l}")
                t_fulls.append(tf)

            # ---- resident constants / state ----
            identf = cpool.tile([128, 128], f32)
            make_identity(nc, identf[:])
            identb = cpool.tile([128, 128], bf16)
            make_identity(nc, identb[:])
            We_sb = cpool.tile([128, H], bf16)
            nc.sync.dma_start(We_sb[:], We_d[:])
            W1_sb = cpool.tile([128, L, 2 * H], bf16)
            nc.sync.dma_start(W1_sb[:], W1_d[:].rearrange("l k n -> k l n"))
            W2_sb = cpool.tile([128, L, 2, H], bf16)
            nc.sync.dma_start(W2_sb[:], W2_d[:].rearrange("l j k n -> k l j n"))
            idx_sb = cpool.tile([128, p.NSLOT // 16], i16)
            nc.sync.dma_start(idx_sb[:], idx_d[:])
            eps_sb = cpool.tile([128, c.NW], f32)
            nc.sync.dma_start(eps_sb[:], eps_d[:])
            bat_sb = cpool.tile([128, c.NW], f32)
            nc.sync.dma_start(bat_sb[:], bat_d[:])
            iota_sb = cpool.tile([128, 128], f32)
            nc.sync.dma_start(iota_sb[:], iota_d[:])
            cnt_sb = cpool.tile([128, ncalls], mybir.dt.int32)
            nc.sync.dma_start(cnt_sb[:], cnt_d[:])
            nq = min(4, c.NCHUNK)
            cnt_regs = [nc.gpsimd.alloc_register(f"cnt_reg{q}")
                        for q in range(nq)]
            epsln_sb = cpool.tile([128, 1], f32)
            nc.vector.memset(epsln_sb[:], EPS_LN)
            t_sb = cpool.tile([128, c.NW, H], tdt)
            h_sb = cpool.tile([128, c.NW, H], f32)
            aff_sb = {}
            if not trivial:
                for k, dd in aff_d.items():
                    if k == "be":
                        t_ = cpool.tile([128, H], f32)
                        nc.sync.dma_start(t_[:], dd[:])
                    else:
                        t_ = cpool.tile([128, L, dd.shape[-1]], f32)
                        nc.sync.dma_start(t_[:], dd[:].rearrange("l p n -> p l n"))
                    aff_sb[k] = t_

            def ln_batch_stats(src_tile, gw, width, tag):
                """Batched LN stats for gw windows: src [128, >=gw, width].
                Returns (rstd, nb) tiles [128, GW, 1]."""
                st4 = smpool.tile([128, c.GW, 6], f32, tag=f"st4{tag}")
                for wg in range(gw):
                    nc.vector.bn_stats(st4[:, wg, :], src_tile[:, wg, :])
                mv4 = smpool.tile([128, c.GW, 2], f32, tag=f"mv4{tag}")
                for wg in range(gw):
                    nc.vector.bn_aggr(mv4[:, wg, :], st4[:, wg, :])
                std4 = smpool.tile([128, c.GW, 1], f32, tag=f"std4{tag}")
                nc.scalar.activation(std4[:, 0:gw, :], mv4[:, 0:gw, 1:2],
                                     AF.Sqrt, bias=epsln_sb[:, 0:1])
                rstd4 = smpool.tile([128, c.GW, 1], f32, tag=f"rstd4{tag}")
                nc.vector.reciprocal(rstd4[:, 0:gw, :], std4[:, 0:gw, :])
                nb4 = smpool.tile([128, c.GW, 1], f32, tag=f"nb4{tag}")
                nc.vector.scalar_tensor_tensor(
                    nb4[:, 0:gw, :], mv4[:, 0:gw, 0:1], -1.0,
                    rstd4[:, 0:gw, :], OP.mult, OP.mult)
                return rstd4, nb4

            def ln_relu_fused(dst, src_ap, gname, bname, lidx, relu, width):
                st = smpool.tile([128, 6], f32, tag="st")
                nc.vector.bn_stats(st[:], src_ap)
                mv = smpool.tile([128, 2], f32, tag="mv")
                nc.vector.bn_aggr(mv[:], st[:])
                std = smpool.tile([128, 1], f32, tag="std")
                nc.scalar.activation(std[:], mv[:, 1:2], AF.Sqrt, bias=epsln_sb[:, 0:1])
                rstd = smpool.tile([128, 1], f32, tag="rstd")
                nc.vector.reciprocal(rstd[:], std[:])
                nb = smpool.tile([128, 1], f32, tag="nb")
                nc.vector.tensor_scalar(nb[:], mv[:, 0:1], rstd[:, 0:1], -1.0,
                                        OP.mult, OP.mult)
                if trivial:
                    nc.scalar.activation(dst, src_ap,
                                         AF.Relu if relu else AF.Identity,
                                         bias=nb[:, 0:1], scale=rstd[:, 0:1])
                else:
                    z = mlppool.tile([128, width], f32, tag=f"lnz{width}")
                    nc.scalar.activation(z[:], src_ap, AF.Identity,
                                         bias=nb[:, 0:1], scale=rstd[:, 0:1])
                    g_ap = aff_sb[gname][:, lidx, :]
                    b_ap = aff_sb[bname][:, lidx, :]
                    nc.vector.tensor_tensor(z[:], z[:], g_ap, op=OP.mult)
                    if relu:
                        nc.vector.tensor_tensor(z[:], z[:], b_ap, op=OP.add)
                        nc.scalar.activation(dst, z[:], AF.Relu)
                    else:
                        nc.vector.tensor_tensor(dst, z[:], b_ap, op=OP.add)

            # ================= encoder =================
            for w in range(c.NW):
                xt_t = mlppool.tile([128, 128], bf16, tag="xt_enc")
                nc.sync.dma_start(xt_t[:], xT[:, w * 128:(w + 1) * 128])
                h0_ps = psB.tile([128, H], f32, tag="tr")
                nc.tensor.matmul(h0_ps[:], xt_t[:], We_sb[:], start=True, stop=True)
                if trivial:
                    nc.vector.tensor_copy(h_sb[:, w, :], h0_ps[:])
                else:
                    nc.vector.tensor_tensor(h_sb[:, w, :], h0_ps[:],
                                            aff_sb["be"][:], op=OP.add)
                nc.scalar.activation(t_sb[:, w, :], h_sb[:, w, :], AF.Identity)

            rg = [list(range(c.NC))]

            def stage_full():
                nc.sync.dma_start(
                    t_stage[:].rearrange("(pp w) h -> pp (w h)", pp=128),
                    t_sb[:].rearrange("pp w h -> pp (w h)"))

            def allgather(l):
                nc.gpsimd.collective_compute(
                    "AllGather", OP.bypass, replica_groups=rg,
                    ins=[t_stage[:]], outs=[t_fulls[l][:]])

            stage_full()
            allgather(0)

            # memset message pool buffers once (stale-SBUF guard: padding
            # slots are never DMA'd; S zeros them, but NaN*0 would poison PE)
            for par in range(4):
                for q in range(c.NCHUNK):
                    mt = msgpool.tile([128, Tmax, 128], tdt, tag=f"msg{q}")
                    nc.vector.memset(mt[:].rearrange("pp t n -> pp (t n)"), 0.0)

            # ================= conv layers =================
            pool_ps = None
            for l in range(L):
                for g in range(c.NGRP):
                    gw = c.grp_windows(g)
                    gtb = int(p.sec_tile_base[g, 0])
                    ntile_g = grp_tiles[g]
                    smb = grp_smat_base[g]
                    nsm_g = grp_smat[g]
                    s_t = spool.tile([128, GS, c.WIN], bf16, tag="s")
                    nc.sync.dma_start(
                        s_t[:, 0:nsm_g, :],
                        S_d[smb:smb + nsm_g, :, :].rearrange("t pp n -> pp t n"))
                    e_t = epool.tile([128, GT, H], bf16, tag="e")
                    nc.sync.dma_start(
                        e_t[:, 0:ntile_g, :],
                        E_d[gtb:gtb + ntile_g, :, :].rearrange("t pp n -> pp t n"))
                    m_bf = mbfpool.tile([128, GT, 128], bf16, tag="mbf")
                    # issue the 4 gathers back-to-back (reg_loads hoisted) so
                    # the 4 SWDGE queue pairs overlap without dispatch stalls
                    msgs = []
                    for q in range(c.NCHUNK):
                        qq = q % nq
                        ci = g * c.NCHUNK + q
                        nc.gpsimd.reg_load(cnt_regs[qq], cnt_sb[0:1, ci:ci + 1])
                    for q in range(c.NCHUNK):
                        Tq = int(p.T[g, q])
                        nidx = Tq * 128
                        toff = int(p.sec_tile_base[g, q]) - gtb
                        msg = msgpool.tile([128, Tmax, 128], tdt, tag=f"msg{q}")
                        msgs.append((msg, Tq, toff))
                        colbase = (gtb + toff) * 8
                        qq = q % nq
                        nc.gpsimd.dma_gather(
                            msg[:, 0:Tq, :],
                            t_fulls[l][q * c.CH:(q + 1) * c.CH, :],
                            idx_sb[:, colbase:colbase + nidx // 16],
                            nidx, cnt_regs[qq], elem_size=H, elem_step=H,
                            single_packet=False, queue_num=qq)
                    for q in range(c.NCHUNK):
                        msg, Tq, toff = msgs[q]
                        msl = msg[:, 0:Tq, :].rearrange("pp t n -> pp (t n)")
                        nc.vector.tensor_tensor(
                            msl, msl,
                            e_t[:, toff:toff + Tq, :].rearrange("pp t n -> pp (t n)"),
                            op=OP.add)
                        # relu on the Scalar engine; eps*deg is folded into
                        # the X-add below (eps_sb), not added per message
                        nc.scalar.activation(
                            m_bf[:, toff:toff + Tq, :].rearrange("pp t n -> pp (t n)"),
                            msl, AF.Relu)
                    # ---- phase A: scatter + X + MM1 for all windows in group
                    y1q = psY.tile([128, c.GW, 2 * H], f32, tag="y1q")
                    for wg in range(gw):
                        w = g * c.GW + wg
                        lst = sched[(g, wg)]
                        agg_ps = psA.tile([128, H], f32, tag="agg")
                        if not lst:
                            nc.vector.memset(agg_ps[:], 0.0)
                        else:
                            for j, (q, t, sm) in enumerate(lst):
                                ti = int(p.sec_tile_base[g, q]) - gtb + t
                                nc.tensor.matmul(
                                    agg_ps[:], s_t[:, sm - smb, :],
                                    m_bf[:, ti, :],
                                    start=(j == 0), stop=(j == len(lst) - 1))
                        X = mlppool.tile([128, H], f32, tag="X")
                        nc.vector.scalar_tensor_tensor(
                            X[:], agg_ps[:], eps_sb[:, w:w + 1], t_sb[:, w, :],
                            OP.add, OP.add)
                        xt_ps = psB.tile([128, 128], f32, tag="tr")
                        nc.tensor.transpose(xt_ps[:], X[:], identf[:])
                        XT = mlppool.tile([128, 128], bf16, tag="XT")
                        nc.scalar.activation(XT[:], xt_ps[:], AF.Identity)
                        nc.tensor.matmul(y1q[:, wg, :], XT[:], W1_sb[:, l, :],
                                         start=True, stop=True)
                        if not trivial:
                            nc.vector.tensor_tensor(y1q[:, wg, :], y1q[:, wg, :],
                                                    aff_sb["b1"][:, l, :], op=OP.add)
                    # ---- phase B: batched LN1 + per-window MLP tail
                    if trivial:
                        rstd1, nb1 = ln_batch_stats(y1q, gw, 2 * H, "a")
                    for wg in range(gw):
                        w = g * c.GW + wg
                        z2 = mlppool.tile([128, 2 * H], f32, tag="z2")
                        if trivial:
                            nc.scalar.activation(z2[:], y1q[:, wg, :], AF.Relu,
                                                 bias=nb1[:, wg, 0:1],
                                                 scale=rstd1[:, wg, 0:1])
                        else:
                            ln_relu_fused(z2[:], y1q[:, wg, :], "g1", "bb1", l,
                                          relu=True, width=2 * H)
                        z2t = mlppool.tile([128, 2, 128], bf16, tag="z2t")
                        for kk in range(2):
                            zt_ps = psB.tile([128, 128], f32, tag="tr")
                            nc.tensor.transpose(zt_ps[:], z2[:, kk * 128:(kk + 1) * 128],
                                                identf[:])
                            nc.scalar.activation(z2t[:, kk, :], zt_ps[:], AF.Identity)
                        y2_ps = psC.tile([128, H], f32, tag="y")
                        for kk in range(2):
                            nc.tensor.matmul(y2_ps[:], z2t[:, kk, :],
                                             W2_sb[:, l, kk, :],
                                             start=(kk == 0), stop=(kk == 1))
                        if l > 0:
                            nc.vector.tensor_tensor(h_sb[:, w, :], y2_ps[:],
                                                    h_sb[:, w, :], op=OP.add)
                        else:
                            nc.vector.tensor_copy(h_sb[:, w, :], y2_ps[:])
                        if not trivial:
                            nc.vector.tensor_tensor(h_sb[:, w, :], h_sb[:, w, :],
                                                    aff_sb["b2"][:, l, :], op=OP.add)
                    # ---- phase C: batched tail LN over the group's windows
                    w0 = g * c.GW
                    if trivial:
                        rstd2, nb2 = ln_batch_stats(
                            h_sb[:, w0:w0 + gw, :], gw, H, "b")
                    for wg in range(gw):
                        w = w0 + wg
                        if l < L - 1:
                            if trivial:
                                nc.scalar.activation(t_sb[:, w, :], h_sb[:, w, :],
                                                     AF.Relu,
                                                     bias=nb2[:, wg, 0:1],
                                                     scale=rstd2[:, wg, 0:1])
                            else:
                                ln_relu_fused(t_sb[:, w, :], h_sb[:, w, :],
                                              "gn", "bn", l, relu=True, width=H)
                        else:
                            hf = mlppool.tile([128, H], bf16, tag="hf")
                            if trivial:
                                nc.scalar.activation(hf[:], h_sb[:, w, :],
                                                     AF.Identity,
                                                     bias=nb2[:, wg, 0:1],
                                                     scale=rstd2[:, wg, 0:1])
                            else:
                                ln_relu_fused(hf[:], h_sb[:, w, :], "gn", "bn", l,
                                              relu=False, width=H)
                            Sg = mlppool.tile([128, 128], bf16, tag="Sg")
                            nc.vector.tensor_scalar(Sg[:], iota_sb[:],
                                                    bat_sb[:, w:w + 1], None,
                                                    OP.is_equal)
                            if pool_ps is None:
                                pool_ps = psP.tile([128, H], f32, tag="pool")
                            nc.tensor.matmul(pool_ps[:], Sg[:], hf[:],
                                             start=(w == 0), stop=(w == c.NW - 1),
                                             skip_group_check=True)
                if l < L - 1:
                    stage_full()
                    allgather(l + 1)
            psb = mlppool.tile([128, H], f32, tag="psb")
            nc.vector.tensor_copy(psb[:], pool_ps[:])
            nc.sync.dma_start(out_d[:], psb[:])

    nc.compile()
    return nc


def make_inputs(cfg, inp):
    c = cfg
    import ml_dtypes
    src = np.asarray(inp['edge_index'][0], np.int64)
    dst = np.asarray(inp['edge_index'][1], np.int64)
    attr = np.asarray(inp['edge_attr'], np.int64)
    batch = np.asarray(inp['batch'], np.int64)
    x = np.asarray(inp['x'], np.float32)
    Etab = np.asarray(inp['Etab'], np.float32)
    We = np.asarray(inp['We'], np.float32)
    W1 = np.asarray(inp['W1'], np.float32)
    W2 = np.asarray(inp['W2'], np.float32)

    trivial = (np.all(np.asarray(inp['be']) == 0) and np.all(np.asarray(inp['b1']) == 0)
               and np.all(np.asarray(inp['g1']) == 1) and np.all(np.asarray(inp['bb1']) == 0)
               and np.all(np.asarray(inp['b2']) == 0) and np.all(np.asarray(inp['gn']) == 1)
               and np.all(np.asarray(inp['bn']) == 0))

    bucketed = [bucket_core(c, core, src, dst, attr) for core in range(c.NC)]
    plan, counts = build_plan(c, bucketed)
    sched, nsmat, grp_smat_base = build_smat_schedule(c, plan, bucketed, counts)

    W2s = np.ascontiguousarray(W2.reshape(L, 2, 128, H))
    iota = np.tile(np.arange(128, dtype=np.float32)[None, :], (128, 1))
    bf = ml_dtypes.bfloat16
    in_maps, metas = [], []
    for core in range(c.NC):
        cd = prep_core(c, plan, sched, core, bucketed, batch, Etab)
        xs = x[core * c.SH:(core + 1) * c.SH]
        xTp = np.zeros((128, c.SHP), np.float32)
        xTp[:, :c.SH] = xs.T
        m = {
            'xT': xTp.astype(bf), 'We': We.astype(bf),
            'W1': W1.astype(bf), 'W2': W2s.astype(bf),
            'idx': cd['idx_buf'],
            'S': cd['S'].astype(bf),
            'E': cd['E'].astype(bf),
            'epsdeg': cd['eps_pm'], 'batchrel': cd['batch_pm'],
            'iota': iota,
            'cnt': np.tile(cd['cnts'][None, :], (128, 1)),
        }
        if not trivial:
            rep = lambda v, wdt: np.tile(np.asarray(v, np.float32)[:, None, :], (1, 128, 1))
            m['gn'] = rep(inp['gn'], H); m['bn'] = rep(inp['bn'], H)
            m['g1'] = rep(inp['g1'], 2 * H); m['bb1'] = rep(inp['bb1'], 2 * H)
            m['b1'] = rep(inp['b1'], 2 * H); m['b2'] = rep(inp['b2'], H)
            m['be'] = np.tile(np.asarray(inp['be'], np.float32)[None, :], (128, 1))
        in_maps.append(m)
        metas.append(cd)
    return in_maps, metas, trivial, plan, sched, nsmat, grp_smat_base


def postprocess(cfg, inp, results, metas):
    c = cfg
    batch = np.asarray(inp['batch'], np.int64)
    sums = np.zeros((c.G, H), np.float32)
    for core in range(c.NC):
        part = results[core]['partial']
        g0 = metas[core]['g0']
        b = batch[core * c.SH:(core + 1) * c.SH]
        gmax = int(b.max()) - g0
        sums[g0:g0 + gmax + 1] += part[:gmax + 1]
    cnt = np.bincount(batch, minlength=c.G).astype(np.float32)
    h_graph = sums / np.maximum(cnt, 1.0)[:, None]
    Wp = np.asarray(inp['Wp'], np.float32)
    bp = np.asarray(inp['bp'], np.float32)
    logits = h_graph @ Wp + bp
    return (1.0 / (1.0 + np.exp(-logits))).reshape(-1).astype(np.float32)


_CACHE = {}


def kernel(**inputs):
    from concourse.bass_utils import run_bass_kernel_spmd
    cfg = CFG()
    in_maps, metas, trivial, plan, sched, nsmat, gsb = make_inputs(cfg, inputs)
    key = ('prog', trivial, plan.key)
    if key not in _CACHE:
        _CACHE[key] = build_program(cfg, plan, sched, nsmat, gsb, trivial)
    nc = _CACHE[key]
    res = run_bass_kernel_spmd(nc, in_maps, core_ids=list(range(cfg.NC)))
    return postprocess(cfg, inputs, res.results, metas)

